# revision 11
# baseline (speedup 1.0000x reference)
"""AttentionBlock (GroupNorm -> 1x1 qkv -> 4-head attention -> 1x1 proj -> residual)
on 8 trn2 NeuronCores, data-parallel over the batch dim (B=8, one element/core).

Layout per core: channel-major [C=512, N=1024] as 4 SBUF tiles of [128, 1024].
All matmuls in float32r (1 cycle/row for free dim >= 256). V is computed
spatial-major directly from the qkv matmul so attention needs no transposes:
  ST[j,i] = sum_d k[d,j] q[d,i]   (K-tile stationary)
  p~T[j,i] = exp(scale*ST)        (ScalarE, PSUM->SBUF)
  rowsum[i] = ones^T @ p~T        (PE)
  PV[d,i] = sum_j v_sp[j,d] p~T[j,i]   -> channel-major attention output
  out = PV * (1/rowsum broadcast)      (softmax divide deferred past PV)
v-bias is folded into proj bias on the host (softmax rows sum to 1).
"""

import numpy as np

B, C, H, W = 8, 512, 32, 32
N = H * W  # 1024
NUM_HEADS = 4
HEAD_DIM = C // NUM_HEADS  # 128
NUM_GROUPS = 32
GROUP_CH = C // NUM_GROUPS  # 16
EPS = 1e-5
NT = C // 128  # 4 channel tiles
NO_QK = 8  # q,k output tiles (1024 channels)
SCALE = 1.0 / float(np.sqrt(HEAD_DIM))
N_CORES = 8


def build_bass():
    import concourse.bacc as bacc
    import concourse.tile as tile
    from concourse import mybir

    f32 = mybir.dt.float32
    f32r = mybir.dt.float32r
    bf16 = mybir.dt.bfloat16
    Act = mybir.ActivationFunctionType
    Alu = mybir.AluOpType
    Ax = mybir.AxisListType

    nc = bacc.Bacc("TRN2", target_bir_lowering=False, debug=False,
                   num_devices=N_CORES)

    d_x = nc.declare_dram_parameter("x", [C, N], f32, isOutput=False)
    d_xb = nc.declare_dram_parameter("xb", [C, N], bf16, isOutput=False)
    d_wt = nc.declare_dram_parameter("qkv_wt", [C, 3 * C], bf16, isOutput=False)
    d_pwt = nc.declare_dram_parameter("proj_wt", [C, C], bf16, isOutput=False)
    d_bqk = nc.declare_dram_parameter("bias_qk", [2 * C], f32, isOutput=False)
    d_beff = nc.declare_dram_parameter("b_eff", [C], f32, isOutput=False)
    d_gam = nc.declare_dram_parameter("norm_w", [C], f32, isOutput=False)
    d_bet = nc.declare_dram_parameter("norm_b", [C], f32, isOutput=False)
    d_sel = nc.declare_dram_parameter("sel", [128, 8], f32, isOutput=False)
    d_selT = nc.declare_dram_parameter("selT", [8, 128], f32, isOutput=False)
    d_ones = nc.declare_dram_parameter("ones", [128, 1], f32r, isOutput=False)
    d_warm = nc.declare_dram_parameter("warm", [128, 512], bf16, isOutput=False)
    d_out = nc.declare_dram_parameter("out", [C, N], f32, isOutput=True)

    with tile.TileContext(nc) as tc:
        with (
            tc.tile_pool(name="persist", bufs=1) as pp,
            tc.tile_pool(name="pt", bufs=16) as p_pt,
            tc.tile_pool(name="rsum", bufs=5) as p_rs,
            tc.tile_pool(name="outp", bufs=2) as p_out,
            tc.tile_pool(name="small", bufs=1) as ps,
            tc.tile_pool(name="psum", bufs=4, space="PSUM") as pm,
        ):
            # ---- loads. sync + scalar HWDGE queues carry the early tensors
            # (balanced); gpsimd SWDGE streams the late ones in background.
            warm = ps.tile([128, 512], bf16, tag="warm", name="warm")
            nc.sync.dma_start(warm[:], d_warm[:, :])
            sel = ps.tile([128, 8], f32, tag="sel", name="sel")
            nc.sync.dma_start(sel[:], d_sel[:, :])
            selT = ps.tile([8, 128], f32, tag="selT", name="selT")
            nc.sync.dma_start(selT[:], d_selT[:, :])
            ones_r = ps.tile([128, 1], f32r, tag="ones_r", name="ones_r")
            nc.sync.dma_start(ones_r[:], d_ones[:, :])
            gam = ps.tile([128, NT], f32, tag="gam", name="gam")
            nc.scalar.dma_start(gam[:], d_gam.rearrange("(a p) -> p a", p=128))
            bet = ps.tile([128, NT], f32, tag="bet", name="bet")
            nc.scalar.dma_start(bet[:], d_bet.rearrange("(a p) -> p a", p=128))
            bqk = ps.tile([128, NO_QK], f32, tag="bqk", name="bqk")
            nc.scalar.dma_start(bqk[:], d_bqk.rearrange("(a p) -> p a", p=128))
            beff = ps.tile([128, NT], f32, tag="beff", name="beff")
            nc.scalar.dma_start(beff[:], d_beff.rearrange("(a p) -> p a", p=128))
            epsv = ps.tile([8, 1], f32, tag="epsv", name="epsv")
            nc.vector.memset(epsv[:], EPS)

            xbs, xs, hs, wts, pwts = [], [], [], [], []
            for t in range(NT):
                xb_t = pp.tile([128, N], bf16, tag=f"xb{t}", name=f"xb{t}")
                eng = nc.scalar if t % 2 == 1 else nc.sync
                eng.dma_start(xb_t[:], d_xb[t * 128:(t + 1) * 128, :])
                xbs.append(xb_t)
            for t in range(NT):
                wt_t = pp.tile([128, 3 * C], bf16, tag=f"wt{t}", name=f"wt{t}")
                eng = nc.scalar if t % 2 == 1 else nc.sync
                eng.dma_start(wt_t[:], d_wt[t * 128:(t + 1) * 128, :])
                wts.append(wt_t)
            # late loads in background on the gpsimd SWDGE queue
            for t in range(NT):
                x_t = pp.tile([128, N], f32, tag=f"x{t}", name=f"x{t}")
                nc.gpsimd.dma_start(x_t[:], d_x[t * 128:(t + 1) * 128, :])
                xs.append(x_t)
            for t in range(NT):
                pwt_t = pp.tile([128, C], bf16, tag=f"pwt{t}", name=f"pwt{t}")
                nc.gpsimd.dma_start(pwt_t[:], d_pwt[t * 128:(t + 1) * 128, :])
                pwts.append(pwt_t)

            # PE warm-up: junk matmul chain (never read) keeps the HAM
            # clock-gate open while inputs stream in.
            junk = pm.tile([128, N], f32, tag="ps", name="junk")

            def junk_mm(n, first=False, last=False):
                for j in range(n):
                    nc.tensor.matmul(junk[0:128, 0:512], warm[:, 0:128],
                                     warm[:, 0:512],
                                     start=(first and j == 0),
                                     stop=(last and j == n - 1),
                                     skip_group_check=True)

            junk_mm(10, first=True)

            # ---- group norm, per-tile (groups never cross 128-ch tiles)
            for t in range(NT):
                h_t = pp.tile([128, N], bf16, tag=f"h{t}", name=f"h{t}")
                hs.append(h_t)
            for t in range(NT):
                st_t = ps.tile([128, 2], f32, tag=f"st{t}", name=f"st{t}")
                nc.vector.reduce_sum(st_t[:, 0:1], xbs[t][:], axis=Ax.X)
                nc.scalar.activation(hs[t][:], xbs[t][:], Act.Square,
                                     accum_out=st_t[:, 1:2])
                psg = pm.tile([128, N], f32, tag="ps", name=f"psg{t}")
                nc.tensor.matmul(psg[0:8, 0:2], sel[:], st_t[:, 0:2],
                                 start=True, stop=True)
                inv_n = 1.0 / float(GROUP_CH * N)
                msr = ps.tile([8, 4], f32, tag=f"msr{t}", name=f"msr{t}")
                nc.scalar.mul(msr[:, 0:1], psg[0:8, 0:1], inv_n)
                nc.scalar.square(msr[:, 3:4], msr[:, 0:1])
                nc.vector.scalar_tensor_tensor(msr[:, 2:3], psg[0:8, 1:2],
                                               inv_n, msr[:, 3:4],
                                               op0=Alu.mult, op1=Alu.subtract)
                nc.scalar.activation(msr[:, 3:4], msr[:, 2:3], Act.Sqrt,
                                     bias=epsv[:, 0:1])
                nc.vector.reciprocal(msr[:, 1:2], msr[:, 3:4])
                pse = pm.tile([128, N], f32, tag="ps", name=f"pse{t}")
                nc.tensor.matmul(pse[:, 0:2], selT[:], msr[:, 0:2],
                                 start=True, stop=True)
                ab_t = ps.tile([128, 3], f32, tag=f"ab{t}", name=f"ab{t}")
                nc.vector.tensor_mul(ab_t[:, 0:1], gam[:, t:t + 1], pse[:, 1:2])
                nc.vector.tensor_mul(ab_t[:, 2:3], pse[:, 0:1], ab_t[:, 0:1])
                nc.vector.tensor_sub(ab_t[:, 1:2], bet[:, t:t + 1], ab_t[:, 2:3])
                if t % 2 == 0:
                    nc.scalar.activation(hs[t][:], xbs[t][:], Act.Identity,
                                         bias=ab_t[:, 1:2], scale=ab_t[:, 0:1])
                else:
                    nc.vector.tensor_scalar(hs[t][:], xbs[t][:],
                                            ab_t[:, 0:1], ab_t[:, 1:2],
                                            op0=Alu.mult, op1=Alu.add)
                junk_mm(5, last=(t == NT - 1))

            # ---- qkv: q,k channel-major [1024 ch, N], bias on DVE
            qks = []
            for ot in range(NO_QK):
                pq = pm.tile([128, N], f32, tag="ps", name=f"pq{ot}")
                for t in range(NT):
                    for half in range(2):
                        nc.tensor.matmul(
                            pq[:, half * 512:(half + 1) * 512],
                            wts[t][:, ot * 128:(ot + 1) * 128],
                            hs[t][:, half * 512:(half + 1) * 512],
                            start=(t == 0), stop=(t == NT - 1))
                qk_t = pp.tile([128, N], bf16, tag=f"qk{ot}", name=f"qk{ot}")
                nc.vector.tensor_scalar_add(qk_t[:], pq[:], bqk[:, ot:ot + 1])
                qks.append(qk_t)

            # ---- v spatial-major [N, 512]
            vs = []
            for nt in range(NO_QK):
                pv_ = pm.tile([128, N], f32, tag="ps", name=f"pvv{nt}")
                for t in range(NT):
                    nc.tensor.matmul(
                        pv_[:, 0:512],
                        hs[t][:, nt * 128:(nt + 1) * 128],
                        wts[t][:, 2 * C:3 * C],
                        start=(t == 0), stop=(t == NT - 1))
                v_t = pp.tile([128, 512], bf16, tag=f"v{nt}", name=f"v{nt}")
                nc.vector.tensor_copy(v_t[:], pv_[:, 0:512])
                vs.append(v_t)

            # ---- attention: software-pipelined heads.
            # Emit ST+exp for head h, then the tail (rowsum tree, PV,
            # normalize) of head h-1, so exps of one head overlap PE work
            # of the previous head.
            all_pts = [None] * NUM_HEADS
            attns = [None] * NUM_HEADS

            def emit_st(h):
                qT = qks[h]
                kT = qks[NUM_HEADS + h]
                pts = []
                for jt in range(NO_QK):
                    pst = pm.tile([128, N], f32, tag="ps", name=f"pst{h}_{jt}")
                    for half in range(2):
                        nc.tensor.matmul(
                            pst[:, half * 512:(half + 1) * 512],
                            kT[:, jt * 128:(jt + 1) * 128],
                            qT[:, half * 512:(half + 1) * 512],
                            start=True, stop=True)
                    pt_jt = p_pt.tile([128, N], bf16, tag="pt",
                                      name=f"pt{h}_{jt}")
                    nc.scalar.activation(pt_jt[:], pst[:], Act.Exp, scale=SCALE)
                    pts.append(pt_jt)
                all_pts[h] = pts

            def emit_tail(h):
                pts = all_pts[h]
                u01 = p_rs.tile([128, N], f32, tag="rs1", name=f"u01_{h}")
                nc.vector.tensor_add(u01[:], pts[0][:], pts[1][:])
                u23 = p_rs.tile([128, N], f32, tag="rs1", name=f"u23_{h}")
                nc.vector.tensor_add(u23[:], pts[2][:], pts[3][:])
                u45 = p_rs.tile([128, N], f32, tag="rs1", name=f"u45_{h}")
                nc.vector.tensor_add(u45[:], pts[4][:], pts[5][:])
                u67 = p_rs.tile([128, N], f32, tag="rs1", name=f"u67_{h}")
                nc.vector.tensor_add(u67[:], pts[6][:], pts[7][:])
                u0123 = p_rs.tile([128, N], f32, tag="rs2", name=f"u0123_{h}")
                nc.vector.tensor_add(u0123[:], u01[:], u23[:])
                u4567 = p_rs.tile([128, N], f32, tag="rs2", name=f"u4567_{h}")
                nc.vector.tensor_add(u4567[:], u45[:], u67[:])
                uallb = p_rs.tile([128, N], f32r, tag="rs2", name=f"uallb_{h}")
                nc.vector.tensor_add(uallb[:], u0123[:], u4567[:])
                ppv = pm.tile([128, N], f32, tag="ps", name=f"ppv{h}")
                for half in range(2):
                    for jt in range(NO_QK):
                        nc.tensor.matmul(
                            ppv[:, half * 512:(half + 1) * 512],
                            vs[jt][:, h * 128:(h + 1) * 128],
                            pts[jt][:, half * 512:(half + 1) * 512],
                            start=(jt == 0), stop=(jt == NO_QK - 1))
                prs = pm.tile([128, N], f32, tag="ps", name=f"prs{h}")
                for half in range(2):
                    nc.tensor.matmul(prs[0:1, half * 512:(half + 1) * 512],
                                     ones_r[:],
                                     uallb[:, half * 512:(half + 1) * 512],
                                     start=True, stop=True)
                rr = ps.tile([1, N], f32, tag="rr", bufs=2, name=f"rr{h}")
                nc.vector.reciprocal_approx_fast(rr[:], prs[0:1, :])
                rb = ps.tile([128, N], f32, tag="rb", bufs=2, name=f"rb{h}")
                nc.gpsimd.partition_broadcast(rb[:], rr[:])
                attn_h = pp.tile([128, N], bf16, tag=f"attn{h}", name=f"attn{h}")
                nc.vector.tensor_mul(attn_h[:], ppv[:], rb[:])
                attns[h] = attn_h

            emit_st(0)
            emit_st(1)
            emit_tail(0)
            emit_st(2)
            emit_tail(1)
            emit_st(3)
            emit_tail(2)
            emit_tail(3)

            # ---- proj + bias + residual
            for ot in range(NT):
                ppr = pm.tile([128, N], f32, tag="ps", name=f"ppr{ot}")
                for h in range(NUM_HEADS):
                    for half in range(2):
                        nc.tensor.matmul(
                            ppr[:, half * 512:(half + 1) * 512],
                            pwts[h][:, ot * 128:(ot + 1) * 128],
                            attns[h][:, half * 512:(half + 1) * 512],
                            start=(h == 0), stop=(h == NUM_HEADS - 1))
                o_t = p_out.tile([128, N], f32, tag="out", name=f"o{ot}")
                nc.vector.scalar_tensor_tensor(o_t[:], ppr[:],
                                               beff[:, ot:ot + 1], xs[ot][:],
                                               op0=Alu.add, op1=Alu.add)
                eng = nc.scalar if ot % 2 == 1 else nc.sync
                eng.dma_start(d_out[ot * 128:(ot + 1) * 128, :], o_t[:])

    nc.compile()
    return nc


def make_in_maps(x, norm_w, norm_b, qkv_w, qkv_b, proj_w, proj_b):
    x = np.asarray(x, dtype=np.float32)
    qkv_w = np.asarray(qkv_w, dtype=np.float32)
    qkv_b = np.asarray(qkv_b, dtype=np.float32)
    proj_w = np.asarray(proj_w, dtype=np.float32)
    proj_b = np.asarray(proj_b, dtype=np.float32)

    import ml_dtypes
    wt = np.ascontiguousarray(qkv_w.T).astype(ml_dtypes.bfloat16)   # [C, 3C]
    pwt = np.ascontiguousarray(proj_w.T).astype(ml_dtypes.bfloat16)  # [C, C]
    b_eff = (proj_b + proj_w @ qkv_b[2 * C:3 * C]).astype(np.float32)
    bias_qk = np.ascontiguousarray(qkv_b[:2 * C])

    p = np.arange(128)
    sel = (p[:, None] // GROUP_CH == np.arange(8)[None, :]).astype(np.float32)
    selT = np.ascontiguousarray(sel.T)

    xs = x.reshape(B, C, N)
    common = {
        "qkv_wt": wt, "proj_wt": pwt, "bias_qk": bias_qk, "b_eff": b_eff,
        "norm_w": np.ascontiguousarray(norm_w, dtype=np.float32),
        "norm_b": np.ascontiguousarray(norm_b, dtype=np.float32),
        "sel": sel, "selT": selT, "ones": np.ones((128, 1), np.float32),
        "warm": np.full((128, 512), 0.5, ml_dtypes.bfloat16),
    }
    return [dict(common, x=np.ascontiguousarray(xs[i]),
                 xb=np.ascontiguousarray(xs[i]).astype(ml_dtypes.bfloat16))
            for i in range(B)]


def run(inputs, trace=False, tmpdir=None):
    from concourse.bass_utils import run_bass_kernel_spmd
    nc = build_bass()
    in_maps = make_in_maps(**inputs)
    res = run_bass_kernel_spmd(nc, in_maps, core_ids=list(range(N_CORES)),
                               trace=trace, tmpdir=tmpdir)
    out = np.stack([res.results[i]["out"] for i in range(N_CORES)])
    return out.reshape(B, C, H, W).astype(np.float32), res


def kernel(**inputs):
    out, _ = run(inputs, trace=False)
    return out


# revision 12
# speedup vs baseline: 1.2042x; 1.2042x over previous
"""AttentionBlock (GroupNorm -> 1x1 qkv -> 4-head attention -> 1x1 proj -> residual)
on 8 trn2 NeuronCores, data-parallel over the batch dim (B=8, one element/core).

Layout per core: channel-major [C=512, N=1024] as 4 SBUF tiles of [128, 1024].
All matmuls in float32r (1 cycle/row for free dim >= 256). V is computed
spatial-major directly from the qkv matmul so attention needs no transposes:
  ST[j,i] = sum_d k[d,j] q[d,i]   (K-tile stationary)
  p~T[j,i] = exp(scale*ST)        (ScalarE, PSUM->SBUF)
  rowsum[i] = ones^T @ p~T        (PE)
  PV[d,i] = sum_j v_sp[j,d] p~T[j,i]   -> channel-major attention output
  out = PV * (1/rowsum broadcast)      (softmax divide deferred past PV)
v-bias is folded into proj bias on the host (softmax rows sum to 1).
"""

import numpy as np

B, C, H, W = 8, 512, 32, 32
N = H * W  # 1024
NUM_HEADS = 4
HEAD_DIM = C // NUM_HEADS  # 128
NUM_GROUPS = 32
GROUP_CH = C // NUM_GROUPS  # 16
EPS = 1e-5
NT = C // 128  # 4 channel tiles
NO_QK = 8  # q,k output tiles (1024 channels)
SCALE = 1.0 / float(np.sqrt(HEAD_DIM))
N_CORES = 8


def build_bass():
    import concourse.bacc as bacc
    import concourse.tile as tile
    from concourse import mybir

    f32 = mybir.dt.float32
    f32r = mybir.dt.float32r
    bf16 = mybir.dt.bfloat16
    Act = mybir.ActivationFunctionType
    Alu = mybir.AluOpType
    Ax = mybir.AxisListType

    nc = bacc.Bacc("TRN2", target_bir_lowering=False, debug=False,
                   num_devices=N_CORES)

    d_x = nc.declare_dram_parameter("x", [C, N], f32, isOutput=False)
    d_xb = nc.declare_dram_parameter("xb", [C, N], bf16, isOutput=False)
    d_wt = nc.declare_dram_parameter("qkv_wt", [C, 3 * C], bf16, isOutput=False)
    d_pwt = nc.declare_dram_parameter("proj_wt", [C, C], bf16, isOutput=False)
    d_cv = nc.declare_dram_parameter("cvec", [128, 20], f32, isOutput=False)
    d_sel = nc.declare_dram_parameter("sel", [128, 8], f32, isOutput=False)
    d_selT = nc.declare_dram_parameter("selT", [8, 128], f32, isOutput=False)
    d_ones = nc.declare_dram_parameter("ones", [128, 1], f32r, isOutput=False)
    d_warm = nc.declare_dram_parameter("warm", [128, 512], bf16, isOutput=False)
    d_out = nc.declare_dram_parameter("out", [C, N], f32, isOutput=True)

    with tile.TileContext(nc) as tc:
        with (
            tc.tile_pool(name="persist", bufs=1) as pp,
            tc.tile_pool(name="pt", bufs=16) as p_pt,
            tc.tile_pool(name="rsum", bufs=5) as p_rs,
            tc.tile_pool(name="outp", bufs=2) as p_out,
            tc.tile_pool(name="small", bufs=1) as ps,
            tc.tile_pool(name="psum", bufs=4, space="PSUM") as pm,
        ):
            # ---- loads. Two HWDGE queues for the early tensors; gpsimd
            # SWDGE streams the late ones (f32 x for residual, proj weights).
            warm = ps.tile([128, 512], bf16, tag="warm", name="warm")
            nc.sync.dma_start(warm[:], d_warm[:, :])
            sel = ps.tile([128, 8], f32, tag="sel", name="sel")
            nc.sync.dma_start(sel[:], d_sel[:, :])
            selT = ps.tile([8, 128], f32, tag="selT", name="selT")
            nc.sync.dma_start(selT[:], d_selT[:, :])
            ones_r = ps.tile([128, 1], f32r, tag="ones_r", name="ones_r")
            nc.sync.dma_start(ones_r[:], d_ones[:, :])
            epsv = ps.tile([8, 1], f32, tag="epsv", name="epsv")
            nc.vector.memset(epsv[:], EPS)

            xbs, xs, hs, wts, pwts = [], [], [], [], []
            for t in range(NT):
                xb_t = pp.tile([128, N], bf16, tag=f"xb{t}", name=f"xb{t}")
                eng = nc.scalar if t >= 2 else nc.sync
                eng.dma_start(xb_t[:], d_xb[t * 128:(t + 1) * 128, :])
                xbs.append(xb_t)
            for t in range(NT):
                wt_t = pp.tile([128, 3 * C], bf16, tag=f"wt{t}", name=f"wt{t}")
                eng = nc.scalar if t >= 2 else nc.sync
                eng.dma_start(wt_t[:], d_wt[t * 128:(t + 1) * 128, :])
                wts.append(wt_t)
            # gam/bet/bias_qk/b_eff pre-packed host-side as one [128,20] f32
            cvec = ps.tile([128, 20], f32, tag="cvec", name="cvec")
            nc.scalar.dma_start(cvec[:], d_cv[:, :])
            gam, bet, bqk, beff = (cvec[:, 0:4], cvec[:, 4:8],
                                   cvec[:, 8:16], cvec[:, 16:20])
            for t in range(NT):
                x_t = pp.tile([128, N], f32, tag=f"x{t}", name=f"x{t}")
                nc.gpsimd.dma_start(x_t[:], d_x[t * 128:(t + 1) * 128, :])
                xs.append(x_t)
            for t in range(NT):
                pwt_t = pp.tile([128, C], bf16, tag=f"pwt{t}", name=f"pwt{t}")
                nc.gpsimd.dma_start(pwt_t[:], d_pwt[t * 128:(t + 1) * 128, :])
                pwts.append(pwt_t)

            # PE warm-up: junk matmul chain (never read) keeps the HAM
            # clock-gate open while inputs stream in.
            junk = pm.tile([128, N], f32, tag="ps", name="junk")

            def junk_mm(n, first=False, last=False):
                for j in range(n):
                    nc.tensor.matmul(junk[0:128, 0:512], warm[:, 0:128],
                                     warm[:, 0:512],
                                     start=(first and j == 0),
                                     stop=(last and j == n - 1),
                                     skip_group_check=True)

            junk_mm(8, first=True)

            # ---- group norm, per-tile (groups never cross 128-ch tiles)
            for t in range(NT):
                h_t = pp.tile([128, N], bf16, tag=f"h{t}", name=f"h{t}")
                hs.append(h_t)
            for t in range(NT):
                st_t = ps.tile([128, 2], f32, tag=f"st{t}", name=f"st{t}")
                nc.vector.reduce_sum(st_t[:, 0:1], xbs[t][:], axis=Ax.X)
                nc.scalar.activation(hs[t][:], xbs[t][:], Act.Square,
                                     accum_out=st_t[:, 1:2])
                psg = pm.tile([128, N], f32, tag="ps", name=f"psg{t}")
                nc.tensor.matmul(psg[0:8, 0:2], sel[:], st_t[:, 0:2],
                                 start=True, stop=True)
                inv_n = 1.0 / float(GROUP_CH * N)
                msr = ps.tile([8, 4], f32, tag=f"msr{t}", name=f"msr{t}")
                nc.scalar.mul(msr[:, 0:1], psg[0:8, 0:1], inv_n)
                nc.scalar.square(msr[:, 3:4], msr[:, 0:1])
                nc.vector.scalar_tensor_tensor(msr[:, 2:3], psg[0:8, 1:2],
                                               inv_n, msr[:, 3:4],
                                               op0=Alu.mult, op1=Alu.subtract)
                nc.scalar.activation(msr[:, 3:4], msr[:, 2:3], Act.Sqrt,
                                     bias=epsv[:, 0:1])
                nc.vector.reciprocal(msr[:, 1:2], msr[:, 3:4])
                pse = pm.tile([128, N], f32, tag="ps", name=f"pse{t}")
                nc.tensor.matmul(pse[:, 0:2], selT[:], msr[:, 0:2],
                                 start=True, stop=True)
                ab_t = ps.tile([128, 3], f32, tag=f"ab{t}", name=f"ab{t}")
                nc.vector.tensor_mul(ab_t[:, 0:1], gam[:, t:t + 1], pse[:, 1:2])
                nc.vector.tensor_mul(ab_t[:, 2:3], pse[:, 0:1], ab_t[:, 0:1])
                nc.vector.tensor_sub(ab_t[:, 1:2], bet[:, t:t + 1], ab_t[:, 2:3])
                if t % 2 == 0:
                    nc.scalar.activation(hs[t][:], xbs[t][:], Act.Identity,
                                         bias=ab_t[:, 1:2], scale=ab_t[:, 0:1])
                else:
                    nc.vector.tensor_scalar(hs[t][:], xbs[t][:],
                                            ab_t[:, 0:1], ab_t[:, 1:2],
                                            op0=Alu.mult, op1=Alu.add)
                junk_mm(4, last=(t == NT - 1))

            qks = [None] * NO_QK
            vs = [None] * NO_QK
            all_pts = [None] * NUM_HEADS
            attns = [None] * NUM_HEADS

            def emit_qkv(ot):
                pq = pm.tile([128, N], f32, tag="ps", name=f"pq{ot}")
                for t in range(NT):
                    for half in range(2):
                        nc.tensor.matmul(
                            pq[:, half * 512:(half + 1) * 512],
                            wts[t][:, ot * 128:(ot + 1) * 128],
                            hs[t][:, half * 512:(half + 1) * 512],
                            start=(t == 0), stop=(t == NT - 1))
                qk_t = pp.tile([128, N], bf16, tag=f"qk{ot}", name=f"qk{ot}")
                nc.vector.tensor_scalar_add(qk_t[:], pq[:], bqk[:, ot:ot + 1])
                qks[ot] = qk_t

            def emit_v(nt):
                pv_ = pm.tile([128, N], f32, tag="ps", name=f"pvv{nt}")
                for t in range(NT):
                    nc.tensor.matmul(
                        pv_[:, 0:512],
                        hs[t][:, nt * 128:(nt + 1) * 128],
                        wts[t][:, 2 * C:3 * C],
                        start=(t == 0), stop=(t == NT - 1))
                v_t = pp.tile([128, 512], bf16, tag=f"v{nt}", name=f"v{nt}")
                nc.vector.tensor_copy(v_t[:], pv_[:, 0:512])
                vs[nt] = v_t

            def emit_st(h):
                qT = qks[h]
                kT = qks[NUM_HEADS + h]
                pts = []
                for jt in range(NO_QK):
                    pst = pm.tile([128, N], f32, tag="ps", name=f"pst{h}_{jt}")
                    for half in range(2):
                        nc.tensor.matmul(
                            pst[:, half * 512:(half + 1) * 512],
                            kT[:, jt * 128:(jt + 1) * 128],
                            qT[:, half * 512:(half + 1) * 512],
                            start=True, stop=True)
                    pt_jt = p_pt.tile([128, N], bf16, tag="pt",
                                      name=f"pt{h}_{jt}")
                    nc.scalar.activation(pt_jt[:], pst[:], Act.Exp, scale=SCALE)
                    pts.append(pt_jt)
                all_pts[h] = pts

            def emit_tail(h):
                pts = all_pts[h]
                # pairwise row-sum tree on DVE; level 1 in bf16 (2x mode)
                u01 = p_rs.tile([128, N], bf16, tag="rs1", name=f"u01_{h}")
                nc.vector.tensor_add(u01[:], pts[0][:], pts[1][:])
                u23 = p_rs.tile([128, N], bf16, tag="rs1", name=f"u23_{h}")
                nc.vector.tensor_add(u23[:], pts[2][:], pts[3][:])
                u45 = p_rs.tile([128, N], bf16, tag="rs1", name=f"u45_{h}")
                nc.vector.tensor_add(u45[:], pts[4][:], pts[5][:])
                u67 = p_rs.tile([128, N], bf16, tag="rs1", name=f"u67_{h}")
                nc.vector.tensor_add(u67[:], pts[6][:], pts[7][:])
                u0123 = p_rs.tile([128, N], f32, tag="rs2", name=f"u0123_{h}")
                nc.vector.tensor_add(u0123[:], u01[:], u23[:])
                u4567 = p_rs.tile([128, N], f32, tag="rs2", name=f"u4567_{h}")
                nc.vector.tensor_add(u4567[:], u45[:], u67[:])
                uallb = p_rs.tile([128, N], f32r, tag="rs2", name=f"uallb_{h}")
                nc.vector.tensor_add(uallb[:], u0123[:], u4567[:])
                ppv = pm.tile([128, N], f32, tag="ps", name=f"ppv{h}")
                for half in range(2):
                    for jt in range(NO_QK):
                        nc.tensor.matmul(
                            ppv[:, half * 512:(half + 1) * 512],
                            vs[jt][:, h * 128:(h + 1) * 128],
                            pts[jt][:, half * 512:(half + 1) * 512],
                            start=(jt == 0), stop=(jt == NO_QK - 1))
                prs = pm.tile([128, N], f32, tag="ps", name=f"prs{h}")
                for half in range(2):
                    nc.tensor.matmul(prs[0:1, half * 512:(half + 1) * 512],
                                     ones_r[:],
                                     uallb[:, half * 512:(half + 1) * 512],
                                     start=True, stop=True)
                rr = ps.tile([1, N], f32, tag="rr", bufs=2, name=f"rr{h}")
                nc.vector.reciprocal_approx_fast(rr[:], prs[0:1, :])
                rb = ps.tile([128, N], f32, tag="rb", bufs=2, name=f"rb{h}")
                nc.gpsimd.partition_broadcast(rb[:], rr[:])
                attn_h = pp.tile([128, N], bf16, tag=f"attn{h}", name=f"attn{h}")
                nc.vector.tensor_mul(attn_h[:], ppv[:], rb[:])
                attns[h] = attn_h

            # interleaved schedule: qkv pairs feed heads as soon as ready
            emit_qkv(0); emit_qkv(4)
            emit_st(0)
            emit_qkv(1); emit_qkv(5)
            emit_st(1)
            for nt in range(NO_QK):
                emit_v(nt)
            emit_tail(0)
            emit_qkv(2); emit_qkv(6)
            emit_st(2)
            emit_tail(1)
            emit_qkv(3); emit_qkv(7)
            emit_st(3)
            emit_tail(2)
            emit_tail(3)

            # ---- proj + bias + residual
            for ot in range(NT):
                ppr = pm.tile([128, N], f32, tag="ps", name=f"ppr{ot}")
                for h in range(NUM_HEADS):
                    for half in range(2):
                        nc.tensor.matmul(
                            ppr[:, half * 512:(half + 1) * 512],
                            pwts[h][:, ot * 128:(ot + 1) * 128],
                            attns[h][:, half * 512:(half + 1) * 512],
                            start=(h == 0), stop=(h == NUM_HEADS - 1))
                o_t = p_out.tile([128, N], f32, tag="out", name=f"o{ot}")
                nc.vector.scalar_tensor_tensor(o_t[:], ppr[:],
                                               beff[:, ot:ot + 1], xs[ot][:],
                                               op0=Alu.add, op1=Alu.add)
                eng = nc.scalar if ot % 2 == 1 else nc.sync
                eng.dma_start(d_out[ot * 128:(ot + 1) * 128, :], o_t[:])

    nc.compile()
    return nc


def make_in_maps(x, norm_w, norm_b, qkv_w, qkv_b, proj_w, proj_b):
    x = np.asarray(x, dtype=np.float32)
    qkv_w = np.asarray(qkv_w, dtype=np.float32)
    qkv_b = np.asarray(qkv_b, dtype=np.float32)
    proj_w = np.asarray(proj_w, dtype=np.float32)
    proj_b = np.asarray(proj_b, dtype=np.float32)

    import ml_dtypes
    wt = np.ascontiguousarray(qkv_w.T).astype(ml_dtypes.bfloat16)   # [C, 3C]
    pwt = np.ascontiguousarray(proj_w.T).astype(ml_dtypes.bfloat16)  # [C, C]
    b_eff = (proj_b + proj_w @ qkv_b[2 * C:3 * C]).astype(np.float32)
    bias_qk = np.ascontiguousarray(qkv_b[:2 * C])

    p = np.arange(128)
    sel = (p[:, None] // GROUP_CH == np.arange(8)[None, :]).astype(np.float32)
    selT = np.ascontiguousarray(sel.T)

    xs = x.reshape(B, C, N)
    cvec = np.zeros((128, 20), np.float32)
    cvec[:, 0:4] = np.asarray(norm_w, np.float32).reshape(4, 128).T
    cvec[:, 4:8] = np.asarray(norm_b, np.float32).reshape(4, 128).T
    cvec[:, 8:16] = bias_qk.reshape(8, 128).T
    cvec[:, 16:20] = b_eff.reshape(4, 128).T
    common = {
        "qkv_wt": wt, "proj_wt": pwt, "cvec": cvec,
        "sel": sel, "selT": selT, "ones": np.ones((128, 1), np.float32),
        "warm": np.full((128, 512), 0.5, ml_dtypes.bfloat16),
    }
    return [dict(common, x=np.ascontiguousarray(xs[i]),
                 xb=np.ascontiguousarray(xs[i]).astype(ml_dtypes.bfloat16))
            for i in range(B)]


def run(inputs, trace=False, tmpdir=None):
    from concourse.bass_utils import run_bass_kernel_spmd
    nc = build_bass()
    in_maps = make_in_maps(**inputs)
    res = run_bass_kernel_spmd(nc, in_maps, core_ids=list(range(N_CORES)),
                               trace=trace, tmpdir=tmpdir)
    out = np.stack([res.results[i]["out"] for i in range(N_CORES)])
    return out.reshape(B, C, H, W).astype(np.float32), res


def kernel(**inputs):
    out, _ = run(inputs, trace=False)
    return out
